# revision 3
# baseline (speedup 1.0000x reference)
"""MultiHeadSectionAttentionImputer on 8 TRN2 NeuronCores (Bass/Tile).

Sharding: the N=6144 existing sections are split across the 8 cores
(768 each). Each core:
  - projects its exist-shard to K,V  (K_loc = X_e @ Wk, V_loc = X_e @ Wv + ones col)
  - projects the full missing set to Q (duplicated across cores; Wq,bq
    pre-scaled by 1/sqrt(d_k) on host)
  - computes scoresT[n,m] per head with a fused 128-deep contraction:
      d' = [q-dims(64) | cooc-bias-dims(64)]  ->  q.k/sqrt(dk) + mb.eb
  - exp() without max subtraction (scores are bounded ~<60; fp32 range ok)
  - partial out^T = attn @ [V | 1]  ->  numerator (64 cols) + denominator
Host combines partial numerators/denominators across cores (softmax over
the full key set), adds bv, and scatters into a copy of ehr_embeddings.

All matmul inputs are float32r (tf32-like, full-rate on PE) except the
attention-weight matmul which uses bf16 (exp output cast, attn in [0, e^60]).
"""

import os
import sys
import numpy as np
from contextlib import ExitStack

sys.path.insert(0, "/opt/trn_rl_repo")

# problem constants (hardcoded; kernel.py must be self-contained)
H = 12          # heads
DK = 64         # head dim
E = 768         # embed dim
TOTAL = H * DK  # 768
M = 2048        # missing sections
N = 6144        # existing sections
S = 8192        # total sections
CORES = 8
NLOC = N // CORES        # 768 keys per core
EC = E // 128            # 6 contraction chunks
NI = NLOC // 128         # 6 key chunks per core
MI = M // 128            # 16 query chunks
PAIRS = H // 2           # 6 head pairs

_CACHE = {}
LAST_EXEC_NS = None
LAST_TRACE_DIR = None


def _build():
    import concourse.bass as bass
    import concourse.tile as tile
    from concourse import bacc, mybir

    F32 = mybir.dt.float32
    F32R = mybir.dt.float32r
    BF16 = mybir.dt.bfloat16
    Exp = mybir.ActivationFunctionType.Exp

    nc = bacc.Bacc("TRN2", target_bir_lowering=False, debug=False)

    # ---- I/O ----
    FP16 = mybir.dt.float16
    xt_m = nc.dram_tensor("xt_m", [E, M], FP16, kind="ExternalInput").ap()
    mbt = nc.dram_tensor("mbt", [H * DK, M], F32R, kind="ExternalInput").ap()
    xt_e = nc.dram_tensor("xt_e", [E, NLOC], FP16, kind="ExternalInput").ap()
    ebt = nc.dram_tensor("ebt", [H * DK, NLOC], F32R, kind="ExternalInput").ap()
    wq = nc.dram_tensor("wq", [E, TOTAL], FP16, kind="ExternalInput").ap()
    wk = nc.dram_tensor("wk", [E, TOTAL], FP16, kind="ExternalInput").ap()
    wv = nc.dram_tensor("wv", [E, TOTAL], FP16, kind="ExternalInput").ap()
    bq = nc.dram_tensor("bq", [128, PAIRS], F32, kind="ExternalInput").ap()
    out_p = nc.dram_tensor("out_p", [H, M, DK + 1], F32, kind="ExternalOutput").ap()

    with tile.TileContext(nc) as tc, ExitStack() as ctx:
        persist = ctx.enter_context(tc.tile_pool(name="persist", bufs=1))
        qpt_pool = ctx.enter_context(tc.tile_pool(name="qpt", bufs=4))
        attn_pool = ctx.enter_context(tc.tile_pool(name="attn", bufs=9))
        osb_pool = ctx.enter_context(tc.tile_pool(name="osb", bufs=4))
        proj_ps = ctx.enter_context(tc.tile_pool(name="proj_ps", bufs=1, space="PSUM"))
        sc_ps = ctx.enter_context(tc.tile_pool(name="sc_ps", bufs=1, space="PSUM"))
        av_ps = ctx.enter_context(tc.tile_pool(name="av_ps", bufs=2, space="PSUM"))

        # ---- stage inputs in SBUF ----
        xtm_sb = []   # 6 x [128, M]
        xte_sb = []   # 6 x [128, NLOC]
        wq_sb = []    # 6 x [128, TOTAL]
        wk_sb = []
        wv_sb = []
        for ec in range(EC):
            t = persist.tile([128, M], FP16, tag=f"xtm{ec}")
            nc.sync.dma_start(t[:], xt_m[ec * 128:(ec + 1) * 128, :])
            xtm_sb.append(t)
            t = persist.tile([128, NLOC], FP16, tag=f"xte{ec}")
            nc.sync.dma_start(t[:], xt_e[ec * 128:(ec + 1) * 128, :])
            xte_sb.append(t)
            for nm, srcap, lst in (("wq", wq, wq_sb), ("wk", wk, wk_sb), ("wv", wv, wv_sb)):
                t = persist.tile([128, TOTAL], FP16, tag=f"{nm}{ec}")
                nc.sync.dma_start(t[:], srcap[ec * 128:(ec + 1) * 128, :])
                lst.append(t)
        bq_sb = persist.tile([128, PAIRS], F32, tag="bq")
        nc.sync.dma_start(bq_sb[:], bq)

        # K'T tiles per head [128, NLOC]: rows = k-dims | eb-dims (parity layout)
        kpt = [persist.tile([128, NLOC], F32R, tag=f"kpt{h}", name=f"kpt{h}") for h in range(H)]
        # V tiles per key chunk [128, H, DK+1] bf16 (ones col at [., ., DK])
        vsb = [persist.tile([128, H, DK + 1], BF16, tag=f"v{ni}", name=f"v{ni}") for ni in range(NI)]

        def emit_kt_proj(p):
            """K^T for head pair p -> kpt[2p], kpt[2p+1]; plus ebt DMAs."""
            h0, h1 = 2 * p, 2 * p + 1
            ps = proj_ps.tile([128, NLOC], F32, tag="proj")
            for ec in range(EC):
                st = (ec == 0)
                sp = (ec == EC - 1)
                nc.tensor.matmul(ps[:, 0:512], lhsT=wk_sb[ec][:, p * 128:(p + 1) * 128],
                                 rhs=xte_sb[ec][:, 0:512], start=st, stop=sp)
                nc.tensor.matmul(ps[:, 512:NLOC], lhsT=wk_sb[ec][:, p * 128:(p + 1) * 128],
                                 rhs=xte_sb[ec][:, 512:NLOC], start=st, stop=sp)
            # even head: k-dims at partitions 0:64 ; odd head: 64:128
            nc.vector.tensor_copy(kpt[h0][0:64, :], ps[0:64, :])
            nc.vector.tensor_copy(kpt[h1][64:128, :], ps[64:128, :])
            nc.sync.dma_start(kpt[h0][64:128, :], ebt[h0 * DK:(h0 + 1) * DK, :])
            nc.sync.dma_start(kpt[h1][0:64, :], ebt[h1 * DK:(h1 + 1) * DK, :])

        def emit_v_proj(ni):
            """V for key chunk ni -> vsb[ni] (bf16, ones col appended)."""
            ps = proj_ps.tile([128, TOTAL], F32, tag="proj")
            for ec in range(EC):
                st = (ec == 0)
                sp = (ec == EC - 1)
                nc.tensor.matmul(ps[:, 0:512], lhsT=xte_sb[ec][:, ni * 128:(ni + 1) * 128],
                                 rhs=wv_sb[ec][:, 0:512], start=st, stop=sp)
                nc.tensor.matmul(ps[:, 512:TOTAL], lhsT=xte_sb[ec][:, ni * 128:(ni + 1) * 128],
                                 rhs=wv_sb[ec][:, 512:TOTAL], start=st, stop=sp)
            nc.vector.tensor_copy(
                vsb[ni][:, :, 0:DK], ps[:].rearrange("p (h d) -> p h d", d=DK)
            )
            nc.vector.memset(vsb[ni][:, :, DK], 1.0)

        def emit_qt_proj(p):
            """Q'^T for head pair p -> two fresh qpt tiles (returned)."""
            h0, h1 = 2 * p, 2 * p + 1
            q0 = qpt_pool.tile([128, M], F32R, tag="qpt")
            q1 = qpt_pool.tile([128, M], F32R, tag="qpt")
            nc.sync.dma_start(q0[64:128, :], mbt[h0 * DK:(h0 + 1) * DK, :])
            nc.sync.dma_start(q1[0:64, :], mbt[h1 * DK:(h1 + 1) * DK, :])
            for mh in range(2):  # m halves of 1024
                ps = proj_ps.tile([128, 1024], F32, tag="proj")
                mo = mh * 1024
                for ec in range(EC):
                    st = (ec == 0)
                    sp = (ec == EC - 1)
                    nc.tensor.matmul(ps[:, 0:512], lhsT=wq_sb[ec][:, p * 128:(p + 1) * 128],
                                     rhs=xtm_sb[ec][:, mo:mo + 512], start=st, stop=sp)
                    nc.tensor.matmul(ps[:, 512:1024], lhsT=wq_sb[ec][:, p * 128:(p + 1) * 128],
                                     rhs=xtm_sb[ec][:, mo + 512:mo + 1024], start=st, stop=sp)
                nc.vector.tensor_scalar_add(
                    q0[0:64, mo:mo + 1024], ps[0:64, :], bq_sb[0:64, p:p + 1])
                nc.vector.tensor_scalar_add(
                    q1[64:128, mo:mo + 1024], ps[64:128, :], bq_sb[64:128, p:p + 1])
            return q0, q1

        def emit_scores_exp(h, qt, ni):
            """scoresT chunk [128 keys, M] + exp -> attnT tile (bf16)."""
            ps = sc_ps.tile([128, M], F32, tag="sc")
            for mj in range(4):
                nc.tensor.matmul(
                    ps[:, mj * 512:(mj + 1) * 512],
                    lhsT=kpt[h][:, ni * 128:(ni + 1) * 128],
                    rhs=qt[:, mj * 512:(mj + 1) * 512],
                    start=True, stop=True)
            at = attn_pool.tile([128, M], BF16, tag="attn")
            nc.scalar.activation(at[:], ps[:], Exp)
            return at

        def emit_av(h, attns, mi):
            """out chunk [128 queries, DK+1] for head h, query chunk mi."""
            ps = av_ps.tile([128, DK + 1], F32, tag="av")
            for ni in range(NI):
                nc.tensor.matmul(
                    ps[:], lhsT=attns[ni][:, mi * 128:(mi + 1) * 128],
                    rhs=vsb[ni][:, h, :],
                    start=(ni == 0), stop=(ni == NI - 1))
            ot = osb_pool.tile([128, DK + 1], F32, tag="osb")
            nc.vector.tensor_copy(ot[:], ps[:])
            nc.sync.dma_start(out_p[h, mi * 128:(mi + 1) * 128, :], ot[:])

        # ---- emission schedule ----
        # fillers: list of closures of PE-side proj work, drained between
        # scores chunks so the PE fills ACT(exp) wait gaps.
        pending_av = []   # av groups of the previous head
        fillers = []

        emit_kt_proj(0)
        qts = {}
        q0, q1 = emit_qt_proj(0)
        qts[0], qts[1] = q0, q1
        # v chunks are emitted as fillers during head 0
        fillers.extend([lambda ni=ni: emit_v_proj(ni) for ni in range(NI)])

        for h in range(H):
            p = h // 2
            if h % 2 == 1 and p + 1 <= PAIRS - 1:
                # schedule next pair's projections as fillers during odd head
                fillers.append(lambda pp=p + 1: emit_kt_proj(pp))
                def qf(pp=p + 1):
                    a, b = emit_qt_proj(pp)
                    qts[2 * pp], qts[2 * pp + 1] = a, b
                fillers.append(qf)
            attns = []
            for ni in range(NI):
                attns.append(emit_scores_exp(h, qts[h], ni))
                # drain up to 3 pending av groups
                for _ in range(3):
                    if pending_av:
                        pending_av.pop(0)()
                # drain one filler per slot
                if fillers:
                    fillers.pop(0)()
            qts[h] = None  # allow qpt slot reuse
            pending_av.extend(
                [lambda hh=h, aa=attns, mm=mi: emit_av(hh, aa, mm) for mi in range(MI)])
        while fillers:
            fillers.pop(0)()
        while pending_av:
            pending_av.pop(0)()

    nc.compile()
    return nc


def _get_nc():
    if "nc" not in _CACHE:
        _CACHE["nc"] = _build()
    return _CACHE["nc"]


def kernel(**inputs):
    global LAST_EXEC_NS, LAST_TRACE_DIR
    from concourse.bass_utils import run_bass_kernel_spmd

    ehr = np.asarray(inputs["ehr_embeddings"], dtype=np.float32)
    mi = np.asarray(inputs["missing_indices"]).astype(np.int64)
    ei = np.asarray(inputs["exist_indices"]).astype(np.int64)
    Wq = np.asarray(inputs["Wq"], dtype=np.float32)
    Wk = np.asarray(inputs["Wk"], dtype=np.float32)
    Wv = np.asarray(inputs["Wv"], dtype=np.float32)
    bq = np.asarray(inputs["bq"], dtype=np.float32)
    bv = np.asarray(inputs["bv"], dtype=np.float32)
    cooc = np.asarray(inputs["cooc_bias"], dtype=np.float32)
    # bk is softmax-shift-invariant (adds a per-query constant to scores);
    # dropped on device, consistent across cores so the combine is exact.

    scale = 1.0 / np.sqrt(np.float32(DK))
    wq_s = np.ascontiguousarray((Wq * scale).astype(np.float16))
    bq_s = np.ascontiguousarray((bq * scale).reshape(PAIRS, 128).T)

    missing_emb = ehr[mi]                       # [M, E]
    xt_m = np.ascontiguousarray(missing_emb.T.astype(np.float16))  # [E, M]
    mbt = np.ascontiguousarray(
        cooc[:, mi, :].transpose(0, 2, 1).reshape(H * DK, M))

    common = {"xt_m": xt_m, "mbt": mbt, "wq": wq_s,
              "wk": np.ascontiguousarray(Wk.astype(np.float16)),
              "wv": np.ascontiguousarray(Wv.astype(np.float16)), "bq": bq_s}
    in_maps = []
    for c in range(CORES):
        eic = ei[c * NLOC:(c + 1) * NLOC]
        xt_e = np.ascontiguousarray(ehr[eic].T.astype(np.float16))  # [E, NLOC]
        ebt = np.ascontiguousarray(
            cooc[:, eic, :].transpose(0, 2, 1).reshape(H * DK, NLOC))
        in_maps.append({**common, "xt_e": xt_e, "ebt": ebt})

    nc = _get_nc()
    trace = os.environ.get("KERNEL_TRACE") == "1"
    kwargs = {}
    if trace:
        import tempfile
        LAST_TRACE_DIR = tempfile.mkdtemp(prefix="kern_trace_")
        kwargs = {"trace": True, "tmpdir": LAST_TRACE_DIR}
        try:
            import ntff_shim
            ntff_shim.install()
        except ImportError:
            pass
    res = run_bass_kernel_spmd(nc, in_maps, list(range(CORES)), **kwargs)
    LAST_EXEC_NS = res.exec_time_ns

    # ---- host combine ----
    num = np.zeros((H, M, DK), dtype=np.float64)
    den = np.zeros((H, M), dtype=np.float64)
    for c in range(CORES):
        op = res.results[c]["out_p"].astype(np.float64)  # [H, M, DK+1]
        num += op[:, :, :DK]
        den += op[:, :, DK]
    out = num / den[:, :, None]                          # [H, M, DK]
    out = out.transpose(1, 0, 2).reshape(M, TOTAL) + bv.astype(np.float64)
    result = ehr.copy()
    result[mi] = out.astype(np.float32)
    return result


# revision 4
# speedup vs baseline: 1.1521x; 1.1521x over previous
"""MultiHeadSectionAttentionImputer on 8 TRN2 NeuronCores (Bass/Tile).

Sharding: the N=6144 existing sections are split across the 8 cores
(768 each). Each core:
  - projects its exist-shard to K,V  (K_loc = X_e @ Wk, V_loc = X_e @ Wv + ones col)
  - projects the full missing set to Q (duplicated across cores; Wq,bq
    pre-scaled by 1/sqrt(d_k) on host)
  - computes scoresT[n,m] per head with a fused 128-deep contraction:
      d' = [q-dims(64) | cooc-bias-dims(64)]  ->  q.k/sqrt(dk) + mb.eb
  - exp() without max subtraction (scores are bounded ~<60; fp32 range ok)
  - partial out^T = attn @ [V | 1]  ->  numerator (64 cols) + denominator
Host combines partial numerators/denominators across cores (softmax over
the full key set), adds bv, and scatters into a copy of ehr_embeddings.

All matmul inputs are float32r (tf32-like, full-rate on PE) except the
attention-weight matmul which uses bf16 (exp output cast, attn in [0, e^60]).
"""

import os
import sys
import numpy as np
from contextlib import ExitStack

sys.path.insert(0, "/opt/trn_rl_repo")

# problem constants (hardcoded; kernel.py must be self-contained)
H = 12          # heads
DK = 64         # head dim
E = 768         # embed dim
TOTAL = H * DK  # 768
M = 2048        # missing sections
N = 6144        # existing sections
S = 8192        # total sections
CORES = 8
NLOC = N // CORES        # 768 keys per core
EC = E // 128            # 6 contraction chunks
NI = NLOC // 128         # 6 key chunks per core
MI = M // 128            # 16 query chunks
PAIRS = H // 2           # 6 head pairs

_CACHE = {}
LAST_EXEC_NS = None
LAST_TRACE_DIR = None


def _build():
    import concourse.bass as bass
    import concourse.tile as tile
    from concourse import bacc, mybir

    F32 = mybir.dt.float32
    F32R = mybir.dt.float32r
    BF16 = mybir.dt.bfloat16
    Exp = mybir.ActivationFunctionType.Exp

    nc = bacc.Bacc("TRN2", target_bir_lowering=False, debug=False)

    # ---- I/O ----
    FP16 = mybir.dt.float16
    xt_m = nc.dram_tensor("xt_m", [E, M], FP16, kind="ExternalInput").ap()
    mbt = nc.dram_tensor("mbt", [H * DK, M], F32R, kind="ExternalInput").ap()
    xt_e = nc.dram_tensor("xt_e", [E, NLOC], FP16, kind="ExternalInput").ap()
    ebt = nc.dram_tensor("ebt", [H * DK, NLOC], F32R, kind="ExternalInput").ap()
    wq = nc.dram_tensor("wq", [E, TOTAL], FP16, kind="ExternalInput").ap()
    wk = nc.dram_tensor("wk", [E, TOTAL], FP16, kind="ExternalInput").ap()
    wv = nc.dram_tensor("wv", [E, TOTAL], FP16, kind="ExternalInput").ap()
    bq = nc.dram_tensor("bq", [128, PAIRS], F32, kind="ExternalInput").ap()
    out_p = nc.dram_tensor("out_p", [H, DK + 1, M], F32, kind="ExternalOutput").ap()

    with tile.TileContext(nc) as tc, ExitStack() as ctx:
        persist = ctx.enter_context(tc.tile_pool(name="persist", bufs=1))
        qpt_pool = ctx.enter_context(tc.tile_pool(name="qpt", bufs=4))
        attn_pool = ctx.enter_context(tc.tile_pool(name="attn", bufs=9))
        osb_pool = ctx.enter_context(tc.tile_pool(name="osb", bufs=4))
        proj_ps = ctx.enter_context(tc.tile_pool(name="proj_ps", bufs=1, space="PSUM"))
        sc_ps = ctx.enter_context(tc.tile_pool(name="sc_ps", bufs=1, space="PSUM"))
        av_ps = ctx.enter_context(tc.tile_pool(name="av_ps", bufs=2, space="PSUM"))

        # ---- stage inputs in SBUF ----
        xtm_sb = []   # 6 x [128, M]
        xte_sb = []   # 6 x [128, NLOC]
        wq_sb = []    # 6 x [128, TOTAL]
        wk_sb = []
        wv_sb = []
        for ec in range(EC):
            t = persist.tile([128, M], FP16, tag=f"xtm{ec}")
            nc.sync.dma_start(t[:], xt_m[ec * 128:(ec + 1) * 128, :])
            xtm_sb.append(t)
            t = persist.tile([128, NLOC], FP16, tag=f"xte{ec}")
            nc.sync.dma_start(t[:], xt_e[ec * 128:(ec + 1) * 128, :])
            xte_sb.append(t)
            for nm, srcap, lst in (("wq", wq, wq_sb), ("wk", wk, wk_sb), ("wv", wv, wv_sb)):
                t = persist.tile([128, TOTAL], FP16, tag=f"{nm}{ec}")
                nc.sync.dma_start(t[:], srcap[ec * 128:(ec + 1) * 128, :])
                lst.append(t)
        bq_sb = persist.tile([128, PAIRS], F32, tag="bq")
        nc.sync.dma_start(bq_sb[:], bq)

        # K'T tiles per head [128, NLOC]: rows = k-dims | eb-dims (parity layout)
        kpt = [persist.tile([128, NLOC], F32R, tag=f"kpt{h}", name=f"kpt{h}") for h in range(H)]
        # V tiles per key chunk [128, H, DK+1] bf16 (ones col at [., ., DK])
        vsb = [persist.tile([128, H, DK + 1], BF16, tag=f"v{ni}", name=f"v{ni}") for ni in range(NI)]

        def emit_kt_proj(p):
            """K^T for head pair p -> kpt[2p], kpt[2p+1]; plus ebt DMAs."""
            h0, h1 = 2 * p, 2 * p + 1
            ps = proj_ps.tile([128, NLOC], F32, tag="proj")
            for ec in range(EC):
                st = (ec == 0)
                sp = (ec == EC - 1)
                nc.tensor.matmul(ps[:, 0:512], lhsT=wk_sb[ec][:, p * 128:(p + 1) * 128],
                                 rhs=xte_sb[ec][:, 0:512], start=st, stop=sp)
                nc.tensor.matmul(ps[:, 512:NLOC], lhsT=wk_sb[ec][:, p * 128:(p + 1) * 128],
                                 rhs=xte_sb[ec][:, 512:NLOC], start=st, stop=sp)
            # even head: k-dims at partitions 0:64 ; odd head: 64:128
            nc.vector.tensor_copy(kpt[h0][0:64, :], ps[0:64, :])
            nc.vector.tensor_copy(kpt[h1][64:128, :], ps[64:128, :])
            nc.sync.dma_start(kpt[h0][64:128, :], ebt[h0 * DK:(h0 + 1) * DK, :])
            nc.sync.dma_start(kpt[h1][0:64, :], ebt[h1 * DK:(h1 + 1) * DK, :])

        def emit_v_proj(ni):
            """V for key chunk ni -> vsb[ni] (bf16, ones col appended)."""
            ps = proj_ps.tile([128, TOTAL], F32, tag="proj")
            for ec in range(EC):
                st = (ec == 0)
                sp = (ec == EC - 1)
                nc.tensor.matmul(ps[:, 0:512], lhsT=xte_sb[ec][:, ni * 128:(ni + 1) * 128],
                                 rhs=wv_sb[ec][:, 0:512], start=st, stop=sp)
                nc.tensor.matmul(ps[:, 512:TOTAL], lhsT=xte_sb[ec][:, ni * 128:(ni + 1) * 128],
                                 rhs=wv_sb[ec][:, 512:TOTAL], start=st, stop=sp)
            nc.vector.tensor_copy(
                vsb[ni][:, :, 0:DK], ps[:].rearrange("p (h d) -> p h d", d=DK)
            )
            nc.vector.memset(vsb[ni][:, :, DK], 1.0)

        def emit_qt_proj(p):
            """Q'^T for head pair p -> two fresh qpt tiles (returned)."""
            h0, h1 = 2 * p, 2 * p + 1
            q0 = qpt_pool.tile([128, M], F32R, tag="qpt")
            q1 = qpt_pool.tile([128, M], F32R, tag="qpt")
            nc.sync.dma_start(q0[64:128, :], mbt[h0 * DK:(h0 + 1) * DK, :])
            nc.sync.dma_start(q1[0:64, :], mbt[h1 * DK:(h1 + 1) * DK, :])
            for mh in range(2):  # m halves of 1024
                ps = proj_ps.tile([128, 1024], F32, tag="proj")
                mo = mh * 1024
                for ec in range(EC):
                    st = (ec == 0)
                    sp = (ec == EC - 1)
                    nc.tensor.matmul(ps[:, 0:512], lhsT=wq_sb[ec][:, p * 128:(p + 1) * 128],
                                     rhs=xtm_sb[ec][:, mo:mo + 512], start=st, stop=sp)
                    nc.tensor.matmul(ps[:, 512:1024], lhsT=wq_sb[ec][:, p * 128:(p + 1) * 128],
                                     rhs=xtm_sb[ec][:, mo + 512:mo + 1024], start=st, stop=sp)
                nc.vector.tensor_scalar_add(
                    q0[0:64, mo:mo + 1024], ps[0:64, :], bq_sb[0:64, p:p + 1])
                nc.vector.tensor_scalar_add(
                    q1[64:128, mo:mo + 1024], ps[64:128, :], bq_sb[64:128, p:p + 1])
            return q0, q1

        def emit_scores_exp(h, qt, ni):
            """scoresT chunk [128 keys, M] + exp -> attnT tile (bf16)."""
            ps = sc_ps.tile([128, M], F32, tag="sc")
            for mj in range(4):
                nc.tensor.matmul(
                    ps[:, mj * 512:(mj + 1) * 512],
                    lhsT=kpt[h][:, ni * 128:(ni + 1) * 128],
                    rhs=qt[:, mj * 512:(mj + 1) * 512],
                    start=True, stop=True)
            at = attn_pool.tile([128, M], BF16, tag="attn")
            nc.scalar.activation(at[:], ps[:], Exp)
            return at

        def emit_av(h, attns, mj):
            """outT chunk [DK+1, 512 queries] for head h, query chunk mj.
            V chunk is the stationary operand (transposed form) so the PE
            streams full 512-wide moving tiles instead of 65-wide ones."""
            ps = av_ps.tile([DK + 1, 512], F32, tag="av")
            mo = mj * 512
            for ni in range(NI):
                nc.tensor.matmul(
                    ps[:], lhsT=vsb[ni][:, h, :],
                    rhs=attns[ni][:, mo:mo + 512],
                    start=(ni == 0), stop=(ni == NI - 1))
            ot = osb_pool.tile([DK + 1, 512], F32, tag="osb")
            nc.vector.tensor_copy(ot[:], ps[:])
            nc.sync.dma_start(out_p[h, :, mo:mo + 512], ot[:])

        # ---- emission schedule ----
        # fillers: list of closures of PE-side proj work, drained between
        # scores chunks so the PE fills ACT(exp) wait gaps.
        pending_av = []   # av groups of the previous head
        fillers = []

        emit_kt_proj(0)
        qts = {}
        q0, q1 = emit_qt_proj(0)
        qts[0], qts[1] = q0, q1
        # v chunks are emitted as fillers during head 0
        fillers.extend([lambda ni=ni: emit_v_proj(ni) for ni in range(NI)])

        for h in range(H):
            p = h // 2
            if h % 2 == 1 and p + 1 <= PAIRS - 1:
                # schedule next pair's projections as fillers during odd head
                fillers.append(lambda pp=p + 1: emit_kt_proj(pp))
                def qf(pp=p + 1):
                    a, b = emit_qt_proj(pp)
                    qts[2 * pp], qts[2 * pp + 1] = a, b
                fillers.append(qf)
            attns = []
            for ni in range(NI):
                attns.append(emit_scores_exp(h, qts[h], ni))
                # drain one pending av group per slot (4 groups/head, 6 slots)
                if pending_av:
                    pending_av.pop(0)()
                # drain one filler per slot
                if fillers:
                    fillers.pop(0)()
            qts[h] = None  # allow qpt slot reuse
            pending_av.extend(
                [lambda hh=h, aa=attns, mm=mj: emit_av(hh, aa, mm) for mj in range(4)])
        while fillers:
            fillers.pop(0)()
        while pending_av:
            pending_av.pop(0)()

    nc.compile()
    return nc


def _get_nc():
    if "nc" not in _CACHE:
        _CACHE["nc"] = _build()
    return _CACHE["nc"]


def kernel(**inputs):
    global LAST_EXEC_NS, LAST_TRACE_DIR
    from concourse.bass_utils import run_bass_kernel_spmd

    ehr = np.asarray(inputs["ehr_embeddings"], dtype=np.float32)
    mi = np.asarray(inputs["missing_indices"]).astype(np.int64)
    ei = np.asarray(inputs["exist_indices"]).astype(np.int64)
    Wq = np.asarray(inputs["Wq"], dtype=np.float32)
    Wk = np.asarray(inputs["Wk"], dtype=np.float32)
    Wv = np.asarray(inputs["Wv"], dtype=np.float32)
    bq = np.asarray(inputs["bq"], dtype=np.float32)
    bv = np.asarray(inputs["bv"], dtype=np.float32)
    cooc = np.asarray(inputs["cooc_bias"], dtype=np.float32)
    # bk is softmax-shift-invariant (adds a per-query constant to scores);
    # dropped on device, consistent across cores so the combine is exact.

    scale = 1.0 / np.sqrt(np.float32(DK))
    wq_s = np.ascontiguousarray((Wq * scale).astype(np.float16))
    bq_s = np.ascontiguousarray((bq * scale).reshape(PAIRS, 128).T)

    missing_emb = ehr[mi]                       # [M, E]
    xt_m = np.ascontiguousarray(missing_emb.T.astype(np.float16))  # [E, M]
    mbt = np.ascontiguousarray(
        cooc[:, mi, :].transpose(0, 2, 1).reshape(H * DK, M))

    common = {"xt_m": xt_m, "mbt": mbt, "wq": wq_s,
              "wk": np.ascontiguousarray(Wk.astype(np.float16)),
              "wv": np.ascontiguousarray(Wv.astype(np.float16)), "bq": bq_s}
    in_maps = []
    for c in range(CORES):
        eic = ei[c * NLOC:(c + 1) * NLOC]
        xt_e = np.ascontiguousarray(ehr[eic].T.astype(np.float16))  # [E, NLOC]
        ebt = np.ascontiguousarray(
            cooc[:, eic, :].transpose(0, 2, 1).reshape(H * DK, NLOC))
        in_maps.append({**common, "xt_e": xt_e, "ebt": ebt})

    nc = _get_nc()
    trace = os.environ.get("KERNEL_TRACE") == "1"
    kwargs = {}
    if trace:
        import tempfile
        LAST_TRACE_DIR = tempfile.mkdtemp(prefix="kern_trace_")
        kwargs = {"trace": True, "tmpdir": LAST_TRACE_DIR}
        try:
            import ntff_shim
            ntff_shim.install()
        except ImportError:
            pass
    res = run_bass_kernel_spmd(nc, in_maps, list(range(CORES)), **kwargs)
    LAST_EXEC_NS = res.exec_time_ns

    # ---- host combine ----
    num = np.zeros((H, DK, M), dtype=np.float64)
    den = np.zeros((H, M), dtype=np.float64)
    for c in range(CORES):
        op = res.results[c]["out_p"].astype(np.float64)  # [H, DK+1, M]
        num += op[:, :DK, :]
        den += op[:, DK, :]
    out = num / den[:, None, :]                          # [H, DK, M]
    out = out.transpose(2, 0, 1).reshape(M, TOTAL) + bv.astype(np.float64)
    result = ehr.copy()
    result[mi] = out.astype(np.float32)
    return result


# revision 7
# speedup vs baseline: 1.1547x; 1.0022x over previous
"""MultiHeadSectionAttentionImputer on 8 TRN2 NeuronCores (Bass/Tile).

Sharding: the N=6144 existing sections are split across the 8 cores
(768 each). Each core:
  - projects its exist-shard to K,V  (K_loc = X_e @ Wk, V_loc = X_e @ Wv + ones col)
  - projects the full missing set to Q (duplicated across cores; Wq,bq
    pre-scaled by 1/sqrt(d_k) on host)
  - computes scoresT[n,m] per head with a fused 128-deep contraction:
      d' = [q-dims(64) | cooc-bias-dims(64)]  ->  q.k/sqrt(dk) + mb.eb
  - exp() without max subtraction (scores are bounded ~<60; fp32 range ok)
  - partial out^T = attn @ [V | 1]  ->  numerator (64 cols) + denominator
Host combines partial numerators/denominators across cores (softmax over
the full key set), adds bv, and scatters into a copy of ehr_embeddings.

All matmul inputs are float32r (tf32-like, full-rate on PE) except the
attention-weight matmul which uses bf16 (exp output cast, attn in [0, e^60]).
"""

import os
import sys
import numpy as np
from contextlib import ExitStack

sys.path.insert(0, "/opt/trn_rl_repo")

# problem constants (hardcoded; kernel.py must be self-contained)
H = 12          # heads
DK = 64         # head dim
E = 768         # embed dim
TOTAL = H * DK  # 768
M = 2048        # missing sections
N = 6144        # existing sections
S = 8192        # total sections
CORES = 8
NLOC = N // CORES        # 768 keys per core
EC = E // 128            # 6 contraction chunks
NI = NLOC // 128         # 6 key chunks per core
MI = M // 128            # 16 query chunks
PAIRS = H // 2           # 6 head pairs

_CACHE = {}
LAST_EXEC_NS = None
LAST_TRACE_DIR = None


def _build():
    import concourse.bass as bass
    import concourse.tile as tile
    from concourse import bacc, mybir

    F32 = mybir.dt.float32
    F32R = mybir.dt.float32r
    BF16 = mybir.dt.bfloat16
    Exp = mybir.ActivationFunctionType.Exp

    nc = bacc.Bacc("TRN2", target_bir_lowering=False, debug=False)

    # ---- I/O ----
    FP16 = mybir.dt.float16
    xt_m = nc.dram_tensor("xt_m", [E, M], FP16, kind="ExternalInput").ap()
    mbt = nc.dram_tensor("mbt", [H * DK, M], F32R, kind="ExternalInput").ap()
    xt_e = nc.dram_tensor("xt_e", [E, NLOC], FP16, kind="ExternalInput").ap()
    ebt = nc.dram_tensor("ebt", [H * DK, NLOC], F32R, kind="ExternalInput").ap()
    wq = nc.dram_tensor("wq", [E, TOTAL], FP16, kind="ExternalInput").ap()
    wk = nc.dram_tensor("wk", [E, TOTAL], FP16, kind="ExternalInput").ap()
    wv = nc.dram_tensor("wv", [E, TOTAL], FP16, kind="ExternalInput").ap()
    bq = nc.dram_tensor("bq", [128, PAIRS], F32, kind="ExternalInput").ap()
    out_p = nc.dram_tensor("out_p", [H, DK + 1, M], F32, kind="ExternalOutput").ap()

    with tile.TileContext(nc) as tc, ExitStack() as ctx:
        persist = ctx.enter_context(tc.tile_pool(name="persist", bufs=1))
        qpt_pool = ctx.enter_context(tc.tile_pool(name="qpt", bufs=4))
        attn_pool = ctx.enter_context(tc.tile_pool(name="attn", bufs=12))
        osb_pool = ctx.enter_context(tc.tile_pool(name="osb", bufs=4))
        proj_ps = ctx.enter_context(tc.tile_pool(name="proj_ps", bufs=1, space="PSUM"))
        sc_ps = ctx.enter_context(tc.tile_pool(name="sc_ps", bufs=1, space="PSUM"))
        av_ps = ctx.enter_context(tc.tile_pool(name="av_ps", bufs=2, space="PSUM"))

        # ---- stage inputs in SBUF ----
        xtm_sb = []   # 6 x [128, M]
        xte_sb = []   # 6 x [128, NLOC]
        wq_sb = []    # 6 x [128, TOTAL]
        wk_sb = []
        wv_sb = []
        # critical-path order: kt0 needs wk+xte(+ebt p0), qt0 needs wq+xtm+mbt
        for ec in range(EC):
            t = persist.tile([128, TOTAL], FP16, tag=f"wk{ec}", name=f"wk{ec}")
            nc.sync.dma_start(t[:], wk[ec * 128:(ec + 1) * 128, :])
            wk_sb.append(t)
            t = persist.tile([128, NLOC], FP16, tag=f"xte{ec}", name=f"xte{ec}")
            nc.sync.dma_start(t[:], xt_e[ec * 128:(ec + 1) * 128, :])
            xte_sb.append(t)
        for ec in range(EC):
            t = persist.tile([128, TOTAL], FP16, tag=f"wq{ec}", name=f"wq{ec}")
            nc.sync.dma_start(t[:], wq[ec * 128:(ec + 1) * 128, :])
            wq_sb.append(t)
            t = persist.tile([128, M], FP16, tag=f"xtm{ec}", name=f"xtm{ec}")
            nc.sync.dma_start(t[:], xt_m[ec * 128:(ec + 1) * 128, :])
            xtm_sb.append(t)
        bq_sb = persist.tile([128, PAIRS], F32, tag="bq")
        nc.sync.dma_start(bq_sb[:], bq)
        for ec in range(EC):
            t = persist.tile([128, TOTAL], FP16, tag=f"wv{ec}", name=f"wv{ec}")
            nc.gpsimd.dma_start(t[:], wv[ec * 128:(ec + 1) * 128, :])
            wv_sb.append(t)

        # K'T tiles per head [128, NLOC]: rows = k-dims | eb-dims (parity layout)
        kpt = [persist.tile([128, NLOC], F32R, tag=f"kpt{h}", name=f"kpt{h}") for h in range(H)]
        # V tiles per key chunk [128, H, DK+1] bf16 (ones col at [., ., DK])
        vsb = [persist.tile([128, H, DK + 1], BF16, tag=f"v{ni}", name=f"v{ni}") for ni in range(NI)]

        def emit_scores_exp(h, qt, ni):
            """scoresT chunk [128 keys, M] + exp -> attnT tile (bf16)."""
            ps = sc_ps.tile([128, M], F32, tag="sc")
            for mj in range(4):
                nc.tensor.matmul(
                    ps[:, mj * 512:(mj + 1) * 512],
                    lhsT=kpt[h][:, ni * 128:(ni + 1) * 128],
                    rhs=qt[:, mj * 512:(mj + 1) * 512],
                    start=True, stop=True)
            at = attn_pool.tile([128, M], BF16, tag="attn")
            nc.scalar.activation(at[:], ps[:], Exp)
            return at

        def emit_av(h, attns, mj):
            """outT chunk [DK+1, 512 queries] for head h, query chunk mj.
            V chunk is the stationary operand (transposed form) so the PE
            streams full 512-wide moving tiles instead of 65-wide ones."""
            ps = av_ps.tile([DK + 1, 512], F32, tag="av")
            mo = mj * 512
            for ni in range(NI):
                nc.tensor.matmul(
                    ps[:], lhsT=vsb[ni][:, h, :],
                    rhs=attns[ni][:, mo:mo + 512],
                    start=(ni == 0), stop=(ni == NI - 1))
            ot = osb_pool.tile([DK + 1, 512], F32, tag="osb")
            nc.vector.tensor_copy(ot[:], ps[:])
            nc.sync.dma_start(out_p[h, :, mo:mo + 512], ot[:])

        # ---- emission schedule ----
        # Unit-queue: small PE work units (~1.4us each) are drained between
        # scores/exp emissions so the PE fills the exp-wait gaps (scores
        # psum has bufs=1, so s(h,ni+1) waits on exp(h,ni)).
        from collections import deque
        units = deque()
        qts = {}
        pair_ready = {0: 0}  # pair -> emitted kt+qt half count (4 = ready)

        def qt_unit(p, mh):
            def f():
                pair_ready[p] = pair_ready.get(p, 0) + 1
                q0, q1 = qts.get(2 * p), qts.get(2 * p + 1)
                if q0 is None:
                    q0 = qpt_pool.tile([128, M], F32R, tag="qpt", name=f"qpt{2*p}")
                    q1 = qpt_pool.tile([128, M], F32R, tag="qpt", name=f"qpt{2*p+1}")
                    h0, h1 = 2 * p, 2 * p + 1
                    nc.sync.dma_start(q0[64:128, :], mbt[h0 * DK:(h0 + 1) * DK, :])
                    nc.sync.dma_start(q1[0:64, :], mbt[h1 * DK:(h1 + 1) * DK, :])
                    qts[2 * p], qts[2 * p + 1] = q0, q1
                emit_qt_half(p, mh, q0, q1)
            return f

        def emit_qt_half(p, mh, q0, q1):
            ps = proj_ps.tile([128, 1024], F32, tag="proj", name="proj_qt")
            mo = mh * 1024
            for ec in range(EC):
                st = (ec == 0)
                sp = (ec == EC - 1)
                nc.tensor.matmul(ps[:, 0:512], lhsT=wq_sb[ec][:, p * 128:(p + 1) * 128],
                                 rhs=xtm_sb[ec][:, mo:mo + 512], start=st, stop=sp)
                nc.tensor.matmul(ps[:, 512:1024], lhsT=wq_sb[ec][:, p * 128:(p + 1) * 128],
                                 rhs=xtm_sb[ec][:, mo + 512:mo + 1024], start=st, stop=sp)
            nc.vector.tensor_scalar_add(
                q0[0:64, mo:mo + 1024], ps[0:64, :], bq_sb[0:64, p:p + 1])
            nc.vector.tensor_scalar_add(
                q1[64:128, mo:mo + 1024], ps[64:128, :], bq_sb[64:128, p:p + 1])

        def kt_unit(p, half):
            def f():
                pair_ready[p] = pair_ready.get(p, 0) + 1
                emit_kt_half(p, half)
            return f

        def emit_kt_half(p, half):
            h0, h1 = 2 * p, 2 * p + 1
            lo, hi = (0, 512) if half == 0 else (512, NLOC)
            ps = proj_ps.tile([128, 512], F32, tag="proj", name="proj_kt")
            for ec in range(EC):
                nc.tensor.matmul(ps[:, 0:hi - lo], lhsT=wk_sb[ec][:, p * 128:(p + 1) * 128],
                                 rhs=xte_sb[ec][:, lo:hi], start=(ec == 0), stop=(ec == EC - 1))
            nc.vector.tensor_copy(kpt[h0][0:64, lo:hi], ps[0:64, 0:hi - lo])
            nc.vector.tensor_copy(kpt[h1][64:128, lo:hi], ps[64:128, 0:hi - lo])
            if half == 0:
                nc.sync.dma_start(kpt[h0][64:128, :], ebt[h0 * DK:(h0 + 1) * DK, :])
                nc.sync.dma_start(kpt[h1][0:64, :], ebt[h1 * DK:(h1 + 1) * DK, :])

        def v_unit(ni, half):
            def f():
                lo, hi = (0, 512) if half == 0 else (512, TOTAL)
                ps = proj_ps.tile([128, 512], F32, tag="proj", name="proj_v")
                for ec in range(EC):
                    nc.tensor.matmul(ps[:, 0:hi - lo],
                                     lhsT=xte_sb[ec][:, ni * 128:(ni + 1) * 128],
                                     rhs=wv_sb[ec][:, lo:hi], start=(ec == 0), stop=(ec == EC - 1))
                hlo, hhi = lo // DK, hi // DK
                nc.vector.tensor_copy(
                    vsb[ni][:, hlo:hhi, 0:DK],
                    ps[:, 0:hi - lo].rearrange("p (h d) -> p h d", d=DK))
                if half == 1:
                    nc.vector.memset(vsb[ni][:, :, DK], 1.0)
            return f

        def av_unit(h, attns, mj):
            def f():
                emit_av(h, attns, mj)
            return f

        # kt pair0 + qt pair0 emitted up front (head 0 critical path)
        emit_kt_half(0, 0)
        emit_kt_half(0, 1)
        pair_ready[0] = 2
        for mh in range(2):
            qt_unit(0, mh)()
        # v units right after (needed by first av drains in head 1)
        for ni in range(NI):
            units.append(v_unit(ni, 0))
            units.append(v_unit(ni, 1))

        slot = 0
        for h in range(H):
            p = h // 2
            if h % 2 == 1 and p + 1 <= PAIRS - 1:
                units.append(kt_unit(p + 1, 0))
                units.append(kt_unit(p + 1, 1))
                for mh in range(2):
                    units.append(qt_unit(p + 1, mh))
            # this pair's K'T and Q'T must be fully emitted before scores
            # read them (a read emitted before its writer would be silently
            # unordered by the tile tracer)
            while pair_ready.get(p, 0) < 4:
                units.popleft()()
            attns = []
            for ni in range(NI):
                attns.append(emit_scores_exp(h, qts[h], ni))
                # drain units: 2 during the fill phase (ACT still ramping)
                # or when the backlog builds up, 1 otherwise
                npump = 2 if (slot < 12 or len(units) > 6) else 1
                for _ in range(npump):
                    if units:
                        units.popleft()()
                slot += 1
            qts[h] = None  # allow qpt slot reuse
            for mj in range(4):
                units.append(av_unit(h, attns, mj))
        while units:
            units.popleft()()

    nc.compile()
    return nc


def _get_nc():
    if "nc" not in _CACHE:
        _CACHE["nc"] = _build()
    return _CACHE["nc"]


def kernel(**inputs):
    global LAST_EXEC_NS, LAST_TRACE_DIR
    from concourse.bass_utils import run_bass_kernel_spmd

    ehr = np.asarray(inputs["ehr_embeddings"], dtype=np.float32)
    mi = np.asarray(inputs["missing_indices"]).astype(np.int64)
    ei = np.asarray(inputs["exist_indices"]).astype(np.int64)
    Wq = np.asarray(inputs["Wq"], dtype=np.float32)
    Wk = np.asarray(inputs["Wk"], dtype=np.float32)
    Wv = np.asarray(inputs["Wv"], dtype=np.float32)
    bq = np.asarray(inputs["bq"], dtype=np.float32)
    bv = np.asarray(inputs["bv"], dtype=np.float32)
    cooc = np.asarray(inputs["cooc_bias"], dtype=np.float32)
    # bk is softmax-shift-invariant (adds a per-query constant to scores);
    # dropped on device, consistent across cores so the combine is exact.

    scale = 1.0 / np.sqrt(np.float32(DK))
    wq_s = np.ascontiguousarray((Wq * scale).astype(np.float16))
    bq_s = np.ascontiguousarray((bq * scale).reshape(PAIRS, 128).T)

    missing_emb = ehr[mi]                       # [M, E]
    xt_m = np.ascontiguousarray(missing_emb.T.astype(np.float16))  # [E, M]
    mbt = np.ascontiguousarray(
        cooc[:, mi, :].transpose(0, 2, 1).reshape(H * DK, M))

    common = {"xt_m": xt_m, "mbt": mbt, "wq": wq_s,
              "wk": np.ascontiguousarray(Wk.astype(np.float16)),
              "wv": np.ascontiguousarray(Wv.astype(np.float16)), "bq": bq_s}
    in_maps = []
    for c in range(CORES):
        eic = ei[c * NLOC:(c + 1) * NLOC]
        xt_e = np.ascontiguousarray(ehr[eic].T.astype(np.float16))  # [E, NLOC]
        ebt = np.ascontiguousarray(
            cooc[:, eic, :].transpose(0, 2, 1).reshape(H * DK, NLOC))
        in_maps.append({**common, "xt_e": xt_e, "ebt": ebt})

    nc = _get_nc()
    trace = os.environ.get("KERNEL_TRACE") == "1"
    kwargs = {}
    if trace:
        import tempfile
        LAST_TRACE_DIR = tempfile.mkdtemp(prefix="kern_trace_")
        kwargs = {"trace": True, "tmpdir": LAST_TRACE_DIR}
        try:
            import ntff_shim
            ntff_shim.install()
        except ImportError:
            pass
    res = run_bass_kernel_spmd(nc, in_maps, list(range(CORES)), **kwargs)
    LAST_EXEC_NS = res.exec_time_ns

    # ---- host combine ----
    num = np.zeros((H, DK, M), dtype=np.float64)
    den = np.zeros((H, M), dtype=np.float64)
    for c in range(CORES):
        op = res.results[c]["out_p"].astype(np.float64)  # [H, DK+1, M]
        num += op[:, :DK, :]
        den += op[:, DK, :]
    out = num / den[:, None, :]                          # [H, DK, M]
    out = out.transpose(2, 0, 1).reshape(M, TOTAL) + bv.astype(np.float64)
    result = ehr.copy()
    result[mi] = out.astype(np.float32)
    return result
